# revision 1
# baseline (speedup 1.0000x reference)
"""CapsNet dynamic-routing kernel for Trainium2 (8 NeuronCores, batch-parallel).

Restructured routing that never materializes u_hat (B=256,D=10,M=32,P=36,I=8,O=16):
  y[b,d,m,i] = sum_p c[b,d,m,p] x[b,m,p,i]        (op A, PE per (b,chunk))
  s[b,d,o]   = sum_{m,i} y[b,d,m,i] W[d,m,o,i]    (op B)
  g[b,d,m,i] = sum_o W[d,m,o,i] v[b,d,o]          (op C)
  b[b,d,m,p]+= sum_i x[b,m,p,i] g[b,d,m,i]        (op D, PE per (b,chunk))

m is split into 11 chunks of 3. b-state/c live on SBUF rows (mc,p)=108 with
cols (chunk,b,d). y/g/z rows are (mc,i)=24 per chunk, packed three chunks per
tile at row offsets 0/32/64 (matmul operands must start at partition 0/32/64)
with zero padding rows 24-31/56-63. All x-derived matmul operands (incl.
block-diagonal forms) are prepared host-side. The final softmax over p is
fused: op A runs on exp(b), then y is scaled by 1/Z from a ones-blockdiag
matmul (Z replicated over i to match y rows).
"""

import numpy as np

B, D, M, P, I, O = 256, 10, 32, 36, 8, 16
NCORES = 8
BC = B // NCORES
NCH = 11
NT = 4                      # triples of chunks: [0-2],[3-5],[6-8],[9-10]
TLEN = [3, 3, 3, 2]
TROWS = [96, 96, 96, 64]    # 32 rows per chunk (24 real + 8 zero pad)
EPS = 1e-7


def _host_prep(xc, Wd):
    """Per-core host-side tensor prep. xc: [BC,M,P,I], Wd: [D,M,O,I]."""
    f32 = np.float32
    xbd = np.zeros((3, P, BC, NCH, 3, I), f32)
    xsum = xc.sum(axis=2) * (1.0 / D)
    out = {}
    for t in range(NT):
        L = TLEN[t]
        rows = TROWS[t]
        t2 = np.zeros((rows, BC, 3, P), f32)
        tw = np.zeros((rows, D, O), f32)
        tcc = np.zeros((O, D, L, 4, I), f32)
        txs = np.zeros((rows, BC), f32)
        for pos in range(L):
            c = 3 * t + pos
            r0 = 32 * pos
            for mc in range(3):
                m = 3 * c + mc
                if m >= M:
                    continue
                xmi = xc[:, m, :, :]                      # [b, p, i]
                xbd[mc, :, :, c, mc, :] = xmi.transpose(1, 0, 2)
                rr = r0 + 8 * mc
                t2[rr:rr + 8, :, mc, :] = xmi.transpose(2, 0, 1)
                tw[rr:rr + 8, :, :] = Wd[:, m, :, :].transpose(2, 0, 1)
                tcc[:, :, pos, mc, :] = Wd[:, m, :, :].transpose(1, 0, 2)
                txs[rr:rr + 8, :] = xsum[:, m, :].T
        out[f"xbd2_t{t}"] = np.ascontiguousarray(t2.reshape(rows, BC * 3 * P))
        out[f"ws_t{t}"] = np.ascontiguousarray(tw.reshape(rows, D * O))
        out[f"wc_t{t}"] = np.ascontiguousarray(tcc.reshape(O, D * L * 32))
        out[f"xs_t{t}"] = txs
    out["xbd"] = np.ascontiguousarray(xbd.reshape(108, BC * NCH * 24))
    ones_bd = np.zeros((3, P, 3, I), f32)
    for mc in range(3):
        ones_bd[mc, :, mc, :] = 1.0
    out["ones_bd"] = ones_bd.reshape(108, 24)
    out["ident"] = np.eye(32, dtype=f32)
    return out


def _build(nc):
    import concourse.mybir as mybir
    import concourse.tile as tile

    f32 = mybir.dt.float32
    AF = mybir.ActivationFunctionType
    AX = mybir.AxisListType
    ALU = mybir.AluOpType

    ins = {}
    specs = [("xbd", [108, BC * NCH * 24]), ("ones_bd", [108, 24]),
             ("ident", [32, 32])]
    for t in range(NT):
        specs += [
            (f"xbd2_t{t}", [TROWS[t], BC * 3 * P]),
            (f"ws_t{t}", [TROWS[t], D * O]),
            (f"wc_t{t}", [O, D * TLEN[t] * 32]),
            (f"xs_t{t}", [TROWS[t], BC]),
        ]
    for name, shape in specs:
        ins[name] = nc.declare_dram_parameter(name, shape, f32, isOutput=False)
    out_d = nc.declare_dram_parameter("out_v", [BC, D * O], f32, isOutput=True)

    with tile.TileContext(nc) as tc:
        with (
            tc.tile_pool(name="const", bufs=1) as cpool,
            tc.tile_pool(name="state", bufs=1) as spool,
            tc.tile_pool(name="small", bufs=2) as mpool,
            tc.tile_pool(name="psA", bufs=4, space="PSUM") as psA,
            tc.tile_pool(name="psS", bufs=2, space="PSUM") as psS,
        ):
            sb = {}
            for name, t in ins.items():
                st = cpool.tile(list(t.shape), f32, name=name, tag=name)
                nc.sync.dma_start(st[:], t[:])
                sb[name] = st

            xbd_r = sb["xbd"][:].rearrange("r (b c k) -> r b c k", b=BC, c=NCH)
            xbd2 = [sb[f"xbd2_t{t}"][:].rearrange("r (b q) -> r b q", b=BC)
                    for t in range(NT)]
            ws = [sb[f"ws_t{t}"][:].rearrange("r (d o) -> r d o", d=D)
                  for t in range(NT)]
            wc = [sb[f"wc_t{t}"][:].rearrange("o (d r) -> o d r", d=D)
                  for t in range(NT)]
            xs = [sb[f"xs_t{t}"] for t in range(NT)]

            bstate = spool.tile([108, NCH * BC * D], f32)
            bst = bstate[:].rearrange("r (c b d) -> r c b d", c=NCH, b=BC)
            ctile = spool.tile([108, NCH * BC * D], f32)
            ct = ctile[:].rearrange("r (c b d) -> r c b d", c=NCH, b=BC)
            ytiles = [spool.tile([TROWS[t], BC * D], f32, tag=f"y{t}",
                                 name=f"y{t}") for t in range(NT)]
            gtiles = [spool.tile([TROWS[t], BC * D], f32, tag=f"g{t}",
                                 name=f"g{t}") for t in range(NT)]
            ztiles = [spool.tile([TROWS[t], BC * D], f32, tag=f"z{t}",
                                 name=f"z{t}") for t in range(NT)]
            for t in range(NT):
                nc.gpsimd.memset(ytiles[t][:], 0.0)   # pad rows must stay 0
                nc.gpsimd.memset(ztiles[t][:], 1.0)   # pad rows must stay 1

            def op_B(src_y, it):
                s_ps = psS.tile([BC, D * O], f32, tag="s", name="s_ps")
                for d in range(D):
                    for t in range(NT):
                        if it == 0:
                            lhsT = xs[t][:]
                        else:
                            lhsT = src_y[t][:].rearrange(
                                "r (b d) -> r d b", d=D)[:, d, :]
                        nc.tensor.matmul(
                            s_ps[:, d * O:(d + 1) * O], lhsT, ws[t][:, d, :],
                            start=(t == 0), stop=(t == NT - 1))
                return s_ps

            def squash(s_ps):
                s_sb = mpool.tile([BC, D * O], f32, tag="ssb", name="s_sb")
                nc.scalar.copy(s_sb[:], s_ps[:])
                sq = mpool.tile([BC, D * O], f32, tag="sq", name="sq")
                nc.vector.tensor_mul(sq[:], s_sb[:], s_sb[:])
                ssum = mpool.tile([BC, D], f32, tag="ssum", name="ssum")
                nc.vector.tensor_reduce(
                    ssum[:], sq[:].rearrange("b (d o) -> b d o", d=D),
                    axis=AX.X, op=ALU.add)
                rt = mpool.tile([BC, D], f32, tag="rt", name="rt")
                nc.vector.tensor_scalar_add(rt[:], ssum[:], EPS)
                nc.scalar.activation(rt[:], rt[:], AF.Sqrt)
                den = mpool.tile([BC, D], f32, tag="den", name="den")
                nc.vector.tensor_scalar_add(den[:], ssum[:], 1.0)
                nc.vector.tensor_mul(den[:], den[:], rt[:])
                rden = mpool.tile([BC, D], f32, tag="rden", name="rden")
                nc.vector.reciprocal(rden[:], den[:])
                sc = mpool.tile([BC, D], f32, tag="sc", name="sc")
                nc.vector.tensor_mul(sc[:], ssum[:], rden[:])
                v = mpool.tile([BC, D * O], f32, tag="v", name="v")
                nc.vector.tensor_mul(
                    v[:].rearrange("b (d o) -> b d o", d=D),
                    s_sb[:].rearrange("b (d o) -> b d o", d=D),
                    sc[:].broadcast_to([BC, D, O]))
                return v

            def op_CD(v, it):
                # stage VT as [o=16, (d,b)] so matmul reads start at partition 0
                vtp = mpool.tile([O, D * 32], f32, tag="vtp", name="vtp")
                for d in range(D):
                    vt_ps = psA.tile([O, 32], f32, tag="ps", name="vt_ps")
                    nc.tensor.transpose(
                        vt_ps[:], v[:, 16 * d:16 * d + 16], sb["ident"][:])
                    nc.vector.tensor_copy(vtp[:, 32 * d:32 * d + 32], vt_ps[:])
                for t in range(NT):
                    L = TLEN[t]
                    g_ps = psA.tile([32 * L, D * 32], f32, tag="ps",
                                    name="g_ps")
                    for d in range(D):
                        nc.tensor.matmul(
                            g_ps[:, 32 * d:32 * d + 32], wc[t][:, d, :],
                            vtp[:, 32 * d:32 * d + 32], start=True, stop=True)
                    nc.vector.tensor_copy(
                        gtiles[t][:].rearrange("r (b d) -> r b d", b=BC),
                        g_ps[:].rearrange("r (d b) -> r b d", d=D))
                for c in range(NCH):
                    t, pos = c // 3, c % 3
                    r0 = 32 * pos
                    d_ps = psA.tile([108, BC * D], f32, tag="ps", name="d_ps")
                    for b in range(BC):
                        nc.tensor.matmul(
                            d_ps[:, D * b:D * b + D],
                            xbd2[t][r0:r0 + 24, b, :],
                            gtiles[t][r0:r0 + 24, D * b:D * b + D],
                            start=True, stop=True)
                    dst = bst[:, c, :, :].rearrange("r b d -> r (b d)")
                    if it == 0:
                        nc.vector.tensor_copy(dst, d_ps[:])
                    else:
                        nc.vector.tensor_add(dst, dst, d_ps[:])

            def op_A(srcr, dst_tiles):
                for c in range(NCH):
                    t, pos = c // 3, c % 3
                    y_ps = psA.tile([24, BC * D], f32, tag="ps", name="y_ps")
                    for b in range(BC):
                        nc.tensor.matmul(
                            y_ps[:, D * b:D * b + D],
                            xbd_r[:, b, c, :],
                            srcr[:, c, b, :],
                            start=True, stop=True)
                    nc.vector.tensor_copy(
                        dst_tiles[t][32 * pos:32 * pos + 24, :], y_ps[:])

            # ---- iteration 1 (c uniform = 1/D, folded into xs) ----
            s_ps = op_B(None, it=0)
            v = squash(s_ps)
            op_CD(v, it=0)

            # ---- iteration 2: softmax over d ----
            nc.scalar.activation(ctile[:], bstate[:], AF.Exp)
            zsum = mpool.tile([108, NCH * BC], f32, tag="zsum", name="zsum")
            nc.vector.tensor_reduce(zsum[:], ct, axis=AX.X, op=ALU.add)
            nc.vector.reciprocal(zsum[:], zsum[:])
            nc.vector.tensor_mul(
                ct, ct,
                zsum[:].rearrange("r (c b) -> r c b", c=NCH)
                .broadcast_to([108, NCH, BC, D]))
            op_A(ct, ytiles)
            s_ps = op_B(ytiles, it=1)
            v = squash(s_ps)
            op_CD(v, it=1)

            # ---- final: softmax over p fused into op A ----
            nc.scalar.activation(ctile[:], bstate[:], AF.Exp)
            for c in range(NCH):
                t, pos = c // 3, c % 3
                z_ps = psA.tile([24, BC * D], f32, tag="ps", name="z_ps")
                nc.tensor.matmul(
                    z_ps[:], sb["ones_bd"][:],
                    ct[:, c, :, :].rearrange("r b d -> r (b d)"),
                    start=True, stop=True)
                nc.vector.tensor_copy(
                    ztiles[t][32 * pos:32 * pos + 24, :], z_ps[:])
            op_A(ct, ytiles)
            for t in range(NT):
                nc.vector.reciprocal(ztiles[t][:], ztiles[t][:])
                nc.vector.tensor_mul(ytiles[t][:], ytiles[t][:], ztiles[t][:])
            s_ps = op_B(ytiles, it=2)
            v = squash(s_ps)
            nc.sync.dma_start(out_d[:], v[:])
    return nc


_CACHE = {}


def kernel(x, W):
    import sys
    if "/opt/trn_rl_repo" not in sys.path:
        sys.path.insert(0, "/opt/trn_rl_repo")
    from concourse import bass_utils

    x = np.asarray(x, np.float32)
    Wd = np.asarray(W, np.float32)[0, :, :, 0]  # [D,M,O,I]
    if "nc" not in _CACHE:
        from concourse import bacc
        nc = _build(bacc.Bacc(None, target_bir_lowering=False))
        nc.compile()
        _CACHE["nc"] = nc
    nc = _CACHE["nc"]
    in_maps = [_host_prep(x[k * BC:(k + 1) * BC], Wd) for k in range(NCORES)]
    res = bass_utils.run_bass_kernel_spmd(nc, in_maps, list(range(NCORES)))
    outs = [res.results[k]["out_v"].reshape(BC, D, O) for k in range(NCORES)]
    return np.concatenate(outs, axis=0)



# revision 2
# speedup vs baseline: 2.4252x; 2.4252x over previous
"""CapsNet dynamic-routing kernel for Trainium2 (8 NeuronCores, batch-parallel).

Restructured routing that never materializes u_hat (B=256,D=10,M=32,P=36,I=8,O=16):
  y[b,d,m,i] = sum_p c[b,d,m,p] x[b,m,p,i]        (op A, PE per (b,chunk))
  s[b,d,o]   = sum_{m,i} y[b,d,m,i] W[d,m,o,i]    (op B)
  g[b,d,m,i] = sum_o W[d,m,o,i] v[b,d,o]          (op C)
  b[b,d,m,p]+= sum_i x[b,p,i] g[b,d,m,i]          (op D, PE per (b,chunk))

m is split into 11 chunks of 3. b-state/c live on SBUF rows (mc,p)=108 with
cols (chunk,b,d). y/g/z rows are (mc,i)=24 per chunk, packed three chunks per
tile at row offsets 0/32/64 (matmul operands must start at partition 0/32/64)
with zero padding rows 24-31/56-63. All x-derived matmul operands (incl.
block-diagonal forms) are prepared host-side. The final softmax over p is
fused: op A runs on exp(b), then y is scaled by 1/Z from a ones-blockdiag
matmul (Z replicated over i to match y rows).

All PE matmul operands are bf16 (fp32 matmuls run as two half-speed LOW/HIGH
passes on TRN2 — bf16 halves the instruction stream and LDWEIGHTS time).
PSUM accumulation, the b-state, and the squash chain stay fp32.
"""

import numpy as np
import ml_dtypes

B, D, M, P, I, O = 256, 10, 32, 36, 8, 16
NCORES = 8
BC = B // NCORES
NCH = 11
NT = 4                      # triples of chunks: [0-2],[3-5],[6-8],[9-10]
TLEN = [3, 3, 3, 2]
TROWS = [96, 96, 96, 64]    # 32 rows per chunk (24 real + 8 zero pad)
EPS = 1e-7

BF16 = ml_dtypes.bfloat16


def _host_prep(xc, Wd):
    """Per-core host-side tensor prep. xc: [BC,M,P,I], Wd: [D,M,O,I]."""
    f32 = np.float32
    xbd = np.zeros((3, P, BC, NCH, 3, I), f32)
    xsum = xc.sum(axis=2) * (1.0 / D)
    out = {}
    for t in range(NT):
        L = TLEN[t]
        rows = TROWS[t]
        t2 = np.zeros((rows, BC, 3, P), f32)
        tw = np.zeros((rows, D, O), f32)
        tcc = np.zeros((O, D, L, 4, I), f32)
        txs = np.zeros((rows, BC), f32)
        for pos in range(L):
            c = 3 * t + pos
            r0 = 32 * pos
            for mc in range(3):
                m = 3 * c + mc
                if m >= M:
                    continue
                xmi = xc[:, m, :, :]                      # [b, p, i]
                xbd[mc, :, :, c, mc, :] = xmi.transpose(1, 0, 2)
                rr = r0 + 8 * mc
                t2[rr:rr + 8, :, mc, :] = xmi.transpose(2, 0, 1)
                tw[rr:rr + 8, :, :] = Wd[:, m, :, :].transpose(2, 0, 1)
                tcc[:, :, pos, mc, :] = Wd[:, m, :, :].transpose(1, 0, 2)
                txs[rr:rr + 8, :] = xsum[:, m, :].T
        out[f"xbd2_t{t}"] = np.ascontiguousarray(
            t2.reshape(rows, BC * 3 * P).astype(BF16))
        out[f"ws_t{t}"] = np.ascontiguousarray(
            tw.reshape(rows, D * O).astype(BF16))
        out[f"wc_t{t}"] = np.ascontiguousarray(
            tcc.reshape(O, D * L * 32).astype(BF16))
        out[f"xs_t{t}"] = txs.astype(BF16)
    out["xbd"] = np.ascontiguousarray(
        xbd.reshape(108, BC * NCH * 24).astype(BF16))
    ones_bd = np.zeros((3, P, 3, I), f32)
    for mc in range(3):
        ones_bd[mc, :, mc, :] = 1.0
    out["ones_bd"] = ones_bd.reshape(108, 24).astype(BF16)
    out["ident"] = np.eye(32, dtype=f32)
    return out


def _build(nc):
    import concourse.mybir as mybir
    import concourse.tile as tile

    f32 = mybir.dt.float32
    bf16 = mybir.dt.bfloat16
    AF = mybir.ActivationFunctionType
    AX = mybir.AxisListType
    ALU = mybir.AluOpType

    ins = {}
    specs = [("xbd", [108, BC * NCH * 24], bf16),
             ("ones_bd", [108, 24], bf16),
             ("ident", [32, 32], f32)]
    for t in range(NT):
        specs += [
            (f"xbd2_t{t}", [TROWS[t], BC * 3 * P], bf16),
            (f"ws_t{t}", [TROWS[t], D * O], bf16),
            (f"wc_t{t}", [O, D * TLEN[t] * 32], bf16),
            (f"xs_t{t}", [TROWS[t], BC], bf16),
        ]
    for name, shape, dt in specs:
        ins[name] = nc.declare_dram_parameter(name, shape, dt, isOutput=False)
    out_d = nc.declare_dram_parameter("out_v", [BC, D * O], f32, isOutput=True)

    with tile.TileContext(nc) as tc:
        with (
            tc.tile_pool(name="const", bufs=1) as cpool,
            tc.tile_pool(name="state", bufs=1) as spool,
            tc.tile_pool(name="small", bufs=2) as mpool,
            tc.tile_pool(name="psA", bufs=4, space="PSUM") as psA,
            tc.tile_pool(name="psS", bufs=2, space="PSUM") as psS,
        ):
            sb = {}
            for name, t in ins.items():
                st = cpool.tile(list(t.shape), t.dtype, name=name, tag=name)
                nc.sync.dma_start(st[:], t[:])
                sb[name] = st

            xbd_r = sb["xbd"][:].rearrange("r (b c k) -> r b c k", b=BC, c=NCH)
            xbd2 = [sb[f"xbd2_t{t}"][:].rearrange("r (b q) -> r b q", b=BC)
                    for t in range(NT)]
            ws = [sb[f"ws_t{t}"][:].rearrange("r (d o) -> r d o", d=D)
                  for t in range(NT)]
            wc = [sb[f"wc_t{t}"][:].rearrange("o (d r) -> o d r", d=D)
                  for t in range(NT)]
            xs = [sb[f"xs_t{t}"] for t in range(NT)]

            bstate = spool.tile([108, NCH * BC * D], f32)
            bst = bstate[:].rearrange("r (c b d) -> r c b d", c=NCH, b=BC)
            ctile = spool.tile([108, NCH * BC * D], bf16)
            ct = ctile[:].rearrange("r (c b d) -> r c b d", c=NCH, b=BC)
            ytiles = [spool.tile([TROWS[t], BC * D], bf16, tag=f"y{t}",
                                 name=f"y{t}") for t in range(NT)]
            gtiles = [spool.tile([TROWS[t], BC * D], bf16, tag=f"g{t}",
                                 name=f"g{t}") for t in range(NT)]
            ztiles = [spool.tile([TROWS[t], BC * D], f32, tag=f"z{t}",
                                 name=f"z{t}") for t in range(NT)]
            for t in range(NT):
                nc.gpsimd.memset(ytiles[t][:], 0.0)   # pad rows must stay 0
                nc.gpsimd.memset(ztiles[t][:], 1.0)   # pad rows must stay 1

            def op_B(src_y, it):
                s_ps = psS.tile([BC, D * O], f32, tag="s", name="s_ps")
                for d in range(D):
                    for t in range(NT):
                        if it == 0:
                            lhsT = xs[t][:]
                        else:
                            lhsT = src_y[t][:].rearrange(
                                "r (b d) -> r d b", d=D)[:, d, :]
                        nc.tensor.matmul(
                            s_ps[:, d * O:(d + 1) * O], lhsT, ws[t][:, d, :],
                            start=(t == 0), stop=(t == NT - 1))
                return s_ps

            def squash(s_ps):
                s_sb = mpool.tile([BC, D * O], f32, tag="ssb", name="s_sb")
                nc.scalar.copy(s_sb[:], s_ps[:])
                sq = mpool.tile([BC, D * O], f32, tag="sq", name="sq")
                nc.vector.tensor_mul(sq[:], s_sb[:], s_sb[:])
                ssum = mpool.tile([BC, D], f32, tag="ssum", name="ssum")
                nc.vector.tensor_reduce(
                    ssum[:], sq[:].rearrange("b (d o) -> b d o", d=D),
                    axis=AX.X, op=ALU.add)
                rt = mpool.tile([BC, D], f32, tag="rt", name="rt")
                nc.vector.tensor_scalar_add(rt[:], ssum[:], EPS)
                nc.scalar.activation(rt[:], rt[:], AF.Sqrt)
                den = mpool.tile([BC, D], f32, tag="den", name="den")
                nc.vector.tensor_scalar_add(den[:], ssum[:], 1.0)
                nc.vector.tensor_mul(den[:], den[:], rt[:])
                rden = mpool.tile([BC, D], f32, tag="rden", name="rden")
                nc.vector.reciprocal(rden[:], den[:])
                sc = mpool.tile([BC, D], f32, tag="sc", name="sc")
                nc.vector.tensor_mul(sc[:], ssum[:], rden[:])
                v = mpool.tile([BC, D * O], f32, tag="v", name="v")
                nc.vector.tensor_mul(
                    v[:].rearrange("b (d o) -> b d o", d=D),
                    s_sb[:].rearrange("b (d o) -> b d o", d=D),
                    sc[:].broadcast_to([BC, D, O]))
                return v

            def op_CD(v, it):
                # stage VT as [o=16, (d,b)] so matmul reads start at partition 0
                vtp = mpool.tile([O, D * 32], bf16, tag="vtp", name="vtp")
                for d in range(D):
                    vt_ps = psA.tile([O, 32], f32, tag="ps", name="vt_ps")
                    nc.tensor.transpose(
                        vt_ps[:], v[:, 16 * d:16 * d + 16], sb["ident"][:])
                    nc.vector.tensor_copy(vtp[:, 32 * d:32 * d + 32], vt_ps[:])
                for t in range(NT):
                    L = TLEN[t]
                    g_ps = psA.tile([32 * L, D * 32], f32, tag="ps",
                                    name="g_ps")
                    for d in range(D):
                        nc.tensor.matmul(
                            g_ps[:, 32 * d:32 * d + 32], wc[t][:, d, :],
                            vtp[:, 32 * d:32 * d + 32], start=True, stop=True)
                    nc.vector.tensor_copy(
                        gtiles[t][:].rearrange("r (b d) -> r b d", b=BC),
                        g_ps[:].rearrange("r (d b) -> r b d", d=D))
                for c in range(NCH):
                    t, pos = c // 3, c % 3
                    r0 = 32 * pos
                    d_ps = psA.tile([108, BC * D], f32, tag="ps", name="d_ps")
                    for b in range(BC):
                        nc.tensor.matmul(
                            d_ps[:, D * b:D * b + D],
                            xbd2[t][r0:r0 + 24, b, :],
                            gtiles[t][r0:r0 + 24, D * b:D * b + D],
                            start=True, stop=True)
                    dst = bst[:, c, :, :].rearrange("r b d -> r (b d)")
                    if it == 0:
                        nc.vector.tensor_copy(dst, d_ps[:])
                    else:
                        nc.vector.tensor_add(dst, dst, d_ps[:])

            def op_A(srcr, dst_tiles):
                for c in range(NCH):
                    t, pos = c // 3, c % 3
                    y_ps = psA.tile([24, BC * D], f32, tag="ps", name="y_ps")
                    for b in range(BC):
                        nc.tensor.matmul(
                            y_ps[:, D * b:D * b + D],
                            xbd_r[:, b, c, :],
                            srcr[:, c, b, :],
                            start=True, stop=True)
                    nc.vector.tensor_copy(
                        dst_tiles[t][32 * pos:32 * pos + 24, :], y_ps[:])

            # ---- iteration 1 (c uniform = 1/D, folded into xs) ----
            s_ps = op_B(None, it=0)
            v = squash(s_ps)
            op_CD(v, it=0)

            # ---- iteration 2: softmax over d ----
            nc.scalar.activation(ctile[:], bstate[:], AF.Exp)
            zsum = mpool.tile([108, NCH * BC], f32, tag="zsum", name="zsum")
            nc.vector.tensor_reduce(zsum[:], ct, axis=AX.X, op=ALU.add)
            nc.vector.reciprocal(zsum[:], zsum[:])
            nc.vector.tensor_mul(
                ct, ct,
                zsum[:].rearrange("r (c b) -> r c b", c=NCH)
                .broadcast_to([108, NCH, BC, D]))
            op_A(ct, ytiles)
            s_ps = op_B(ytiles, it=1)
            v = squash(s_ps)
            op_CD(v, it=1)

            # ---- final: softmax over p fused into op A ----
            nc.scalar.activation(ctile[:], bstate[:], AF.Exp)
            for c in range(NCH):
                t, pos = c // 3, c % 3
                z_ps = psA.tile([24, BC * D], f32, tag="ps", name="z_ps")
                nc.tensor.matmul(
                    z_ps[:], sb["ones_bd"][:],
                    ct[:, c, :, :].rearrange("r b d -> r (b d)"),
                    start=True, stop=True)
                nc.vector.tensor_copy(
                    ztiles[t][32 * pos:32 * pos + 24, :], z_ps[:])
            op_A(ct, ytiles)
            for t in range(NT):
                nc.vector.reciprocal(ztiles[t][:], ztiles[t][:])
                nc.vector.tensor_mul(ytiles[t][:], ytiles[t][:], ztiles[t][:])
            s_ps = op_B(ytiles, it=2)
            v = squash(s_ps)
            nc.sync.dma_start(out_d[:], v[:])
    return nc


_CACHE = {}


def kernel(x, W):
    import sys
    if "/opt/trn_rl_repo" not in sys.path:
        sys.path.insert(0, "/opt/trn_rl_repo")
    from concourse import bass_utils

    x = np.asarray(x, np.float32)
    Wd = np.asarray(W, np.float32)[0, :, :, 0]  # [D,M,O,I]
    if "nc" not in _CACHE:
        from concourse import bacc
        nc = _build(bacc.Bacc(None, target_bir_lowering=False))
        nc.compile()
        _CACHE["nc"] = nc
    nc = _CACHE["nc"]
    in_maps = [_host_prep(x[k * BC:(k + 1) * BC], Wd) for k in range(NCORES)]
    res = bass_utils.run_bass_kernel_spmd(nc, in_maps, list(range(NCORES)))
    outs = [res.results[k]["out_v"].reshape(BC, D, O) for k in range(NCORES)]
    return np.concatenate(outs, axis=0)


# revision 6
# speedup vs baseline: 3.1463x; 1.2973x over previous
"""CapsNet dynamic-routing kernel for Trainium2 (8 NeuronCores, batch-parallel).

Restructured routing that never materializes u_hat (B=256,D=10,M=32,P=36,I=8,O=16):
  y[b,d,m,i] = sum_p c[b,d,m,p] x[b,m,p,i]        (op A, PE per (b,chunk))
  s[b,d,o]   = sum_{m,i} y[b,d,m,i] W[d,m,o,i]    (op B)
  g[b,d,m,i] = sum_o W[d,m,o,i] v[b,d,o]          (op C)
  b[b,d,m,p]+= sum_i x[b,p,i] g[b,d,m,i]          (op D, PE per (b,chunk))

m is split into 11 chunks of 3, grouped in 4 triples t (chunks 3t..3t+L-1).
b-state/c live on SBUF rows (mc,p)=108, one tile per triple with cols
(pos,b,d). y/g/z rows are (mc,i)=24 per chunk at row offsets 0/32/64 with
zero padding rows. All x-derived matmul operands are prepared host-side.

PE operands are bf16 (fp32 matmuls cost 2x LDWEIGHTS+MATMUL passes); PSUM
accumulation, b-state and squash stay fp32. The scalar engine only ever uses
Copy/Square/Ln/Exp (all in one activation-table set; sqrt = exp(-.5 ln)
avoids table reloads). Softmaxes are processed per-triple so the PE pipeline
overlaps the vector work, PSUM->SBUF casts go to scalar/gpsimd, and the
softmax divides use the fast DVE reciprocal approximation.
"""

import numpy as np
import ml_dtypes

B, D, M, P, I, O = 256, 10, 32, 36, 8, 16
NCORES = 8
BC = B // NCORES
NCH = 11
NT = 4                      # triples of chunks: [0-2],[3-5],[6-8],[9-10]
TLEN = [3, 3, 3, 2]
TROWS = [96, 96, 96, 64]    # 32 rows per chunk (24 real + 8 zero pad)
EPS = 1e-7

BF16 = ml_dtypes.bfloat16


def _host_prep(xc, Wd):
    """Per-core host-side tensor prep. xc: [BC,M,P,I], Wd: [D,M,O,I]."""
    f32 = np.float32
    xbd = np.zeros((3, P, BC, NCH, 3, I), f32)
    xsum = xc.sum(axis=2) * (1.0 / D)
    out = {}
    for t in range(NT):
        L = TLEN[t]
        rows = TROWS[t]
        t2 = np.zeros((rows, BC, 3, P), f32)
        tw = np.zeros((rows, D, O), f32)
        tcc = np.zeros((O, D, L, 4, I), f32)
        txs = np.zeros((rows, BC), f32)
        for pos in range(L):
            c = 3 * t + pos
            r0 = 32 * pos
            for mc in range(3):
                m = 3 * c + mc
                if m >= M:
                    continue
                xmi = xc[:, m, :, :]                      # [b, p, i]
                xbd[mc, :, :, c, mc, :] = xmi.transpose(1, 0, 2)
                rr = r0 + 8 * mc
                t2[rr:rr + 8, :, mc, :] = xmi.transpose(2, 0, 1)
                tw[rr:rr + 8, :, :] = Wd[:, m, :, :].transpose(2, 0, 1)
                tcc[:, :, pos, mc, :] = Wd[:, m, :, :].transpose(1, 0, 2)
                txs[rr:rr + 8, :] = xsum[:, m, :].T
        out[f"xbd2_t{t}"] = np.ascontiguousarray(
            t2.reshape(rows, BC * 3 * P).astype(BF16))
        out[f"ws_t{t}"] = np.ascontiguousarray(
            tw.reshape(rows, D * O).astype(BF16))
        out[f"wc_t{t}"] = np.ascontiguousarray(
            tcc.reshape(O, D * L * 32).astype(BF16))
        out[f"xs_t{t}"] = txs.astype(BF16)
    out["xbd"] = np.ascontiguousarray(
        xbd.reshape(108, BC * NCH * 24).astype(BF16))
    ones_bd = np.zeros((3, P, 3, I), f32)
    for mc in range(3):
        ones_bd[mc, :, mc, :] = 1.0
    out["ones_bd"] = ones_bd.reshape(108, 24).astype(BF16)
    out["ident"] = np.eye(32, dtype=f32)
    return out


def _build(nc):
    import concourse.mybir as mybir
    import concourse.tile as tile

    f32 = mybir.dt.float32
    bf16 = mybir.dt.bfloat16
    AF = mybir.ActivationFunctionType
    AX = mybir.AxisListType
    ALU = mybir.AluOpType

    ins = {}
    # declaration order == DMA issue order: small it0-critical tensors first,
    # the big op_A operand (xbd) last.
    specs = []
    for t in range(NT):
        specs += [(f"xs_t{t}", [TROWS[t], BC], bf16),
                  (f"ws_t{t}", [TROWS[t], D * O], bf16)]
    specs += [("ident", [32, 32], f32)]
    for t in range(NT):
        specs += [(f"wc_t{t}", [O, D * TLEN[t] * 32], bf16)]
    specs += [("ones_bd", [108, 24], bf16)]
    for t in range(NT):
        specs += [(f"xbd2_t{t}", [TROWS[t], BC * 3 * P], bf16)]
    specs += [("xbd", [108, BC * NCH * 24], bf16)]
    for name, shape, dt in specs:
        ins[name] = nc.declare_dram_parameter(name, shape, dt, isOutput=False)
    out_d = nc.declare_dram_parameter("out_v", [BC, D * O], f32, isOutput=True)

    with tile.TileContext(nc) as tc:
        with (
            tc.tile_pool(name="const", bufs=1) as cpool,
            tc.tile_pool(name="state", bufs=1) as spool,
            tc.tile_pool(name="small", bufs=2) as mpool,
            tc.tile_pool(name="psA", bufs=6, space="PSUM") as psA,
            tc.tile_pool(name="psS", bufs=2, space="PSUM") as psS,
        ):
            sb = {}
            for name, t in ins.items():
                st = cpool.tile(list(t.shape), t.dtype, name=name, tag=name)
                nc.sync.dma_start(st[:], t[:])
                sb[name] = st

            xbd_r = sb["xbd"][:].rearrange("r (b c k) -> r b c k", b=BC, c=NCH)
            xbd2 = [sb[f"xbd2_t{t}"][:].rearrange("r (b q) -> r b q", b=BC)
                    for t in range(NT)]
            ws = [sb[f"ws_t{t}"][:].rearrange("r (d o) -> r d o", d=D)
                  for t in range(NT)]
            wc = [sb[f"wc_t{t}"][:].rearrange("o (d r) -> o d r", d=D)
                  for t in range(NT)]
            xs = [sb[f"xs_t{t}"] for t in range(NT)]

            # per-triple routing state: [108, L*BC*D]
            bstate = [spool.tile([108, TLEN[t] * BC * D], f32,
                                 name=f"bst{t}", tag=f"bst{t}")
                      for t in range(NT)]
            bst = [bstate[t][:].rearrange("r (c b d) -> r c b d",
                                          c=TLEN[t], b=BC)
                   for t in range(NT)]
            ctile = [spool.tile([108, TLEN[t] * BC * D], bf16,
                                name=f"ct{t}", tag=f"ct{t}")
                     for t in range(NT)]
            ct = [ctile[t][:].rearrange("r (c b d) -> r c b d",
                                        c=TLEN[t], b=BC)
                  for t in range(NT)]
            ytiles = [spool.tile([TROWS[t], BC * D], bf16, tag=f"y{t}",
                                 name=f"y{t}") for t in range(NT)]
            gtiles = [spool.tile([TROWS[t], BC * D], bf16, tag=f"g{t}",
                                 name=f"g{t}") for t in range(NT)]
            ztiles = [spool.tile([TROWS[t], BC * D], f32, tag=f"z{t}",
                                 name=f"z{t}") for t in range(NT)]
            for t in range(NT):
                nc.gpsimd.memset(ytiles[t][:], 0.0)   # pad rows must stay 0
                nc.gpsimd.memset(ztiles[t][:], 1.0)   # pad rows must stay 1

            def op_B(src_y, it):
                s_ps = psS.tile([BC, D * O], f32, tag="s", name="s_ps")
                for d in range(D):
                    for t in range(NT):
                        if it == 0:
                            lhsT = xs[t][:]
                        else:
                            lhsT = src_y[t][:].rearrange(
                                "r (b d) -> r d b", d=D)[:, d, :]
                        nc.tensor.matmul(
                            s_ps[:, d * O:(d + 1) * O], lhsT, ws[t][:, d, :],
                            start=(t == 0), stop=(t == NT - 1))
                return s_ps

            def squash(s_ps):
                # v = s * ssum / ((1+ssum) sqrt(ssum+eps)); sqrt via exp/ln
                # (keeps the scalar engine inside one activation-table set).
                s_sb = mpool.tile([BC, D * O], f32, tag="ssb", name="s_sb")
                nc.scalar.copy(s_sb[:], s_ps[:])
                sq = mpool.tile([BC, D * O], f32, tag="sq", name="sq")
                nc.scalar.activation(sq[:], s_ps[:], AF.Square)
                ssum = mpool.tile([BC, D], f32, tag="ssum", name="ssum")
                nc.vector.tensor_reduce(
                    ssum[:], sq[:].rearrange("b (d o) -> b d o", d=D),
                    axis=AX.X, op=ALU.add)
                se = mpool.tile([BC, D], f32, tag="se", name="se")
                nc.vector.tensor_scalar_add(se[:], ssum[:], EPS)
                lt = mpool.tile([BC, D], f32, tag="lt", name="lt")
                nc.scalar.activation(lt[:], se[:], AF.Ln)
                rs = mpool.tile([BC, D], f32, tag="rs", name="rs")
                nc.scalar.activation(rs[:], lt[:], AF.Exp, scale=-0.5)
                den = mpool.tile([BC, D], f32, tag="den", name="den")
                nc.vector.tensor_scalar_add(den[:], ssum[:], 1.0)
                rden = mpool.tile([BC, D], f32, tag="rden", name="rden")
                nc.vector.reciprocal_approx_fast(rden[:], den[:])
                sc = mpool.tile([BC, D], f32, tag="sc", name="sc")
                nc.vector.tensor_mul(sc[:], ssum[:], rden[:])
                nc.vector.tensor_mul(sc[:], sc[:], rs[:])
                v = mpool.tile([BC, D * O], f32, tag="v", name="v")
                nc.vector.tensor_mul(
                    v[:].rearrange("b (d o) -> b d o", d=D),
                    s_sb[:].rearrange("b (d o) -> b d o", d=D),
                    sc[:].broadcast_to([BC, D, O]))
                return v

            def op_CD(v, it):
                # stage VT as [o=16, (d,b)] so matmul reads start at partition 0
                vtp = mpool.tile([O, D * 32], bf16, tag="vtp", name="vtp")
                for d in range(D):
                    vt_ps = psA.tile([O, 32], f32, tag="ps", name="vt_ps")
                    nc.tensor.transpose(
                        vt_ps[:], v[:, 16 * d:16 * d + 16], sb["ident"][:])
                    nc.scalar.copy(vtp[:, 32 * d:32 * d + 32], vt_ps[:])
                for t in range(NT):
                    L = TLEN[t]
                    g_ps = psA.tile([32 * L, D * 32], f32, tag="ps",
                                    name="g_ps")
                    for d in range(D):
                        nc.tensor.matmul(
                            g_ps[:, 32 * d:32 * d + 32], wc[t][:, d, :],
                            vtp[:, 32 * d:32 * d + 32], start=True, stop=True)
                    nc.scalar.copy(
                        gtiles[t][:].rearrange("r (b d) -> r b d", b=BC),
                        g_ps[:].rearrange("r (d b) -> r b d", d=D))
                for c in range(NCH):
                    t, pos = c // 3, c % 3
                    r0 = 32 * pos
                    d_ps = psA.tile([108, BC * D], f32, tag="ps", name="d_ps")
                    for b in range(BC):
                        nc.tensor.matmul(
                            d_ps[:, D * b:D * b + D],
                            xbd2[t][r0:r0 + 24, b, :],
                            gtiles[t][r0:r0 + 24, D * b:D * b + D],
                            start=True, stop=True)
                    dst = bst[t][:, pos, :, :].rearrange("r b d -> r (b d)")
                    if it == 0:
                        nc.vector.tensor_copy(dst, d_ps[:])
                    else:
                        nc.vector.tensor_add(dst, dst, d_ps[:])

            def op_A_group(t, dst_tiles):
                # op A for the chunks of triple t; y casts on gpsimd so the
                # vector queue stays free for the softmax chains.
                L = TLEN[t]
                for pos in range(L):
                    c = 3 * t + pos
                    y_ps = psA.tile([24, BC * D], f32, tag="ps", name="y_ps")
                    for b in range(BC):
                        nc.tensor.matmul(
                            y_ps[:, D * b:D * b + D],
                            xbd_r[:, b, c, :],
                            ct[t][:, pos, b, :],
                            start=True, stop=True)
                    nc.scalar.copy(
                        dst_tiles[t][32 * pos:32 * pos + 24, :], y_ps[:])

            def softmax_d_group(t):
                # softmax over d on triple t's b-state -> ct[t] (bf16);
                # the exp itself is hoisted by the caller so the scalar
                # queue is not blocked behind PE-dependent casts.
                L = TLEN[t]
                zs = mpool.tile([108, L * BC], f32, tag=f"zs{t}",
                                name=f"zs{t}")
                nc.vector.tensor_reduce(zs[:], ct[t], axis=AX.X, op=ALU.add)
                zr = mpool.tile([108, L * BC], f32, tag=f"zr{t}",
                                name=f"zr{t}")
                nc.vector.reciprocal_approx_fast(zr[:], zs[:])
                nc.vector.tensor_mul(
                    ct[t], ct[t],
                    zr[:].rearrange("r (c b) -> r c b", c=L)
                    .broadcast_to([108, L, BC, D]))

            # ---- iteration 1 (c uniform = 1/D, folded into xs) ----
            s_ps = op_B(None, it=0)
            v = squash(s_ps)
            op_CD(v, it=0)

            # ---- iteration 2: softmax over d, pipelined per triple ----
            for t in range(NT):
                nc.scalar.activation(ctile[t][:], bstate[t][:], AF.Exp)
            for t in range(NT):
                softmax_d_group(t)
                op_A_group(t, ytiles)
            s_ps = op_B(ytiles, it=1)
            v = squash(s_ps)
            op_CD(v, it=1)

            # ---- final: softmax over p fused into op A, per triple ----
            for t in range(NT):
                nc.scalar.activation(ctile[t][:], bstate[t][:], AF.Exp)
            for t in range(NT):
                L = TLEN[t]
                for pos in range(L):
                    z_ps = psA.tile([24, BC * D], f32, tag="ps", name="z_ps")
                    nc.tensor.matmul(
                        z_ps[:], sb["ones_bd"][:],
                        ct[t][:, pos, :, :].rearrange("r b d -> r (b d)"),
                        start=True, stop=True)
                    nc.scalar.copy(
                        ztiles[t][32 * pos:32 * pos + 24, :], z_ps[:])
                op_A_group(t, ytiles)
                zrt = spool.tile([TROWS[t], BC * D], f32, tag=f"zr_t{t}",
                                 name=f"zr_t{t}")
                nc.vector.reciprocal_approx_fast(zrt[:], ztiles[t][:])
                nc.gpsimd.tensor_mul(ytiles[t][:], ytiles[t][:], zrt[:])
            s_ps = op_B(ytiles, it=2)
            v = squash(s_ps)
            nc.sync.dma_start(out_d[:], v[:])
    return nc


_CACHE = {}


def kernel(x, W):
    import sys
    if "/opt/trn_rl_repo" not in sys.path:
        sys.path.insert(0, "/opt/trn_rl_repo")
    from concourse import bass_utils

    x = np.asarray(x, np.float32)
    Wd = np.asarray(W, np.float32)[0, :, :, 0]  # [D,M,O,I]
    if "nc" not in _CACHE:
        from concourse import bacc
        nc = _build(bacc.Bacc(None, target_bir_lowering=False))
        nc.compile()
        _CACHE["nc"] = nc
    nc = _CACHE["nc"]
    in_maps = [_host_prep(x[k * BC:(k + 1) * BC], Wd) for k in range(NCORES)]
    res = bass_utils.run_bass_kernel_spmd(nc, in_maps, list(range(NCORES)))
    outs = [res.results[k]["out_v"].reshape(BC, D, O) for k in range(NCORES)]
    return np.concatenate(outs, axis=0)
